# revision 18
# baseline (speedup 1.0000x reference)
"""Depthwise causal Conv1d (k=4) + SiLU on 8 Trainium2 NeuronCores.

Problem: x [4, 4096, 2048] f32, w [2048, 4] f32,
out[b, t, d] = silu(sum_j w[d, j] * x[b, t - 3 + j, d])   (zero-padded left).

Sharding: 8 cores = 4 batches x 2 channel-halves. Depthwise conv is
independent per channel, so channel sharding needs no halo exchange.

Layout: each core receives its shard host-transposed to [channels, time]
(channels on SBUF partitions); per-channel weights are per-partition
scalars and causal shifts are free-dim AP offsets. Both DRAM tensors are
HALF-MAJOR so every [128, ~2048] DMA row is dense: xt row (half*DH+ch)
holds x[ch, half*2048-3 : half*2048+2048] (3-col halo duplicated on the
host), ot row (half*DH+ch) holds out[ch, half*2048 : +2048].

Precision: x and the output are host-cast fp16 (halves HBM traffic both
ways); products and adds stay fp16 (PE accumulates fp32 in PSUM); SiLU
computes fp32-internally on ACT. End-to-end relative error ~5e-4.

v10 design, tuned against NTFF profiles of v1-v9:
 - Per-core budget is DMA: ~16.9 MB over a ~435 GB/s fabric (R+W
   combined); loads and stores stream concurrently on separate queues
   (HWDGE via SyncE / SWDGE via GpSimd) nearly the whole run. Shipping
   the diag stationaries from HBM was tried (v8/v9) and lost ~4us to
   the bigger+slower load stream, so they are built on device.
 - Work is cut into [128ch, 2048t] chunks. 5 blocks ride the
   TensorEngine as one back-to-back stream: diag(w_j) matmuls, taps
   outer, 4-tap PSUM accumulation ([128,2048] PSUM tiles x2 = all 8
   banks), SiLU straight out of PSUM. 3 blocks ride DVE: 4
   shift-rebased tensor_scalar products + pair-packed adds.
 - The 20 diag builds ([128,128] mask x per-partition weight) hide in
   engine dead time: block 1 on DVE before its first chain, block 6 on
   ACT after the warmup silu, blocks 3/5/7 on DVE between early chains
   (each lands just before the PE stream needs it). GpSimd was tried
   and is useless for this (~2.5us per op vs 250-477ns on DVE/ACT).
 - The first x row of each stream loads in two pieces and the first
   chunks compute at 1024 so both engines start ~2us earlier; the last
   chunks are split at 1024 so the drain is fine-grained.
 - ACT/store emission order is hand-matched to production order so the
   in-order engines never head-of-line block.
"""

import sys
import types

import numpy as np

import concourse.bass as bass
import concourse.bacc as bacc
import concourse.mybir as mybir
from concourse.tile import TileContext
from concourse.bass_utils import run_bass_kernel_spmd


def _ensure_ntff_hook():
    """bass_utils imports antenv.axon_hooks when BASS_TRACE is set; that
    module is absent on this image. Install a shim so tracing works when
    possible and degrades gracefully (instead of crashing) when not."""
    try:
        import antenv.axon_hooks  # noqa: F401

        return
    except ImportError:
        pass
    try:
        import antenv

        hook = None
        try:
            if "/root/.axon_site" not in sys.path:
                sys.path.insert(0, "/root/.axon_site")
            from trn_agent_boot.trn_boot import _ntff_profile_via_ctypes

            hook = _ntff_profile_via_ctypes("/opt/axon/libaxon_pjrt.so")
        except Exception:
            hook = None
        mod = types.ModuleType("antenv.axon_hooks")
        mod._hook = hook
        mod.get_axon_ntff_profile_hook = lambda: mod._hook
        mod.set_axon_ntff_profile_hook = lambda h: setattr(mod, "_hook", h)
        sys.modules["antenv.axon_hooks"] = mod
        antenv.axon_hooks = mod
    except Exception:
        pass


_ensure_ntff_hook()

B, L, D = 4, 4096, 2048
K = 4
PAD = K - 1
N_CORES = 8
DH = D // 2            # channels per core
NBLK = DH // 128       # 128-partition channel blocks per core
C = 2048               # time chunk (half of L)
XROW = C + PAD         # 2051 data cols per xt row
XPITCH = 2064          # xt row pitch (fp16 elems), 32B-aligned

MID_DT = mybir.dt.float16
PE_BLKS = [1, 6, 3, 5, 7]   # blocks on the TensorEngine (stream order)
DVE_BLKS = [0, 2, 4]        # blocks on DVE
_PE_IDX = {b: i for i, b in enumerate(PE_BLKS)}

_cache = {}


def _build_bass():
    nc = bacc.Bacc()
    # half-major inputs/outputs: row (half*DH + ch)
    xt = nc.dram_tensor("xt", [2 * DH, XPITCH], MID_DT, kind="ExternalInput")
    wt = nc.dram_tensor("wt", [128, NBLK * K], mybir.dt.float32, kind="ExternalInput")
    # [128,128] identity mask; diag(w_j) stationaries are built on device
    dg = nc.dram_tensor("dg", [128, 128], MID_DT, kind="ExternalInput")
    ot = nc.dram_tensor("ot", [2 * DH, C], MID_DT, kind="ExternalOutput")
    f32 = mybir.dt.float32

    with TileContext(nc) as tc:
        with tc.tile_pool(name="pool", bufs=2) as pool, \
             tc.tile_pool(name="psum", bufs=2, space="PSUM") as psum_pool:
            # Warmup: a tiny Silu forces the silu activation-table set to
            # load during the initial DMA wait; it is the only table load
            # in the whole kernel.
            warm = pool.tile([128, 2], MID_DT, tag="warm", bufs=1)
            nc.vector.memset(warm[:], 0.0)
            nc.scalar.activation(warm[:], warm[:], mybir.ActivationFunctionType.Silu)

            w = pool.tile([128, NBLK * K], f32, tag="w", bufs=1)
            nc.sync.dma_start(out=w[:], in_=wt[:, :])
            mask = pool.tile([128, 128], MID_DT, tag="mask", bufs=1)
            nc.sync.dma_start(out=mask[:], in_=dg[:, :])

            dgw = pool.tile([128, len(PE_BLKS) * K * 128], MID_DT, tag="dgw", bufs=1)

            def diag_slot(blk, j):
                bi = _PE_IDX[blk]
                c0 = (bi * K + j) * 128
                return dgw[:, c0 : c0 + 128]

            def diag_build(eng, blk):
                for j in range(K):
                    m = (nc.scalar.mul if eng == "act"
                         else nc.vector.tensor_scalar_mul)
                    m(diag_slot(blk, j), mask[:],
                      w[:, blk * K + j : blk * K + j + 1])

            def wj(blk, j):
                return w[:, blk * K + j : blk * K + j + 1]

            xts = {}

            def load(blk, half, c0=0, cn=XROW):
                r0 = half * DH + blk * 128
                x = pool.tile([128, cn - c0 + 1], MID_DT, tag="x", bufs=10)
                nc.sync.dma_start(
                    out=x[:, 0 : cn - c0], in_=xt[r0 : r0 + 128, c0:cn]
                )
                xts.setdefault((blk, half), []).append((c0, cn, x))

            def xap(blk, half, a, b):
                """AP for cols [a, b) of block (blk, half)'s padded row."""
                for c0, cn, x in xts[(blk, half)]:
                    if a >= c0 and b <= cn:
                        return x[:, a - c0 : b - c0]
                raise KeyError((blk, half, a, b))

            ps_of = {}

            def pe_mm(blk, half, off=0, tl=C):
                """Fill one [128, tl] PSUM tile with the 4-tap conv of one
                (sub)chunk. Taps outer: one stationary per 4 matmuls."""
                ps = psum_pool.tile([128, tl], f32, tag="ps", bufs=2)
                ps_of[(blk, half, off)] = ps
                for j in range(K):
                    lw = diag_slot(blk, j)
                    for c in range(tl // 512):
                        h0 = off + c * 512 + j
                        nc.tensor.matmul(
                            ps[:, c * 512 : (c + 1) * 512],
                            lw,
                            xap(blk, half, h0, h0 + 512),
                            start=(j == 0),
                            stop=(j == K - 1),
                        )

            qe_of = {}

            def dve_chain(blk, half, off=0, tl=C):
                """Elementwise (sub)chunk: 4 shift-rebased products, pair-
                packed adds (qe=[q0|q2] + qo=[q1|q3], then fold into qe0)."""
                qe = pool.tile([128, 2, tl], MID_DT, tag="qe", bufs=2)
                qo = pool.tile([128, 2, tl], MID_DT, tag="qo", bufs=2)
                qe_of[(blk, half, off)] = qe
                o = off
                nc.vector.tensor_scalar_mul(
                    qe[:, 0, :], xap(blk, half, o, o + tl), wj(blk, 0))
                nc.vector.tensor_scalar_mul(
                    qo[:, 0, :], xap(blk, half, o + 1, o + 1 + tl), wj(blk, 1))
                nc.vector.tensor_scalar_mul(
                    qe[:, 1, :], xap(blk, half, o + 2, o + 2 + tl), wj(blk, 2))
                nc.vector.tensor_scalar_mul(
                    qo[:, 1, :], xap(blk, half, o + 3, o + 3 + tl), wj(blk, 3))
                nc.vector.tensor_add(qe[:, :, :], qe[:, :, :], qo[:, :, :])
                nc.vector.tensor_add(qe[:, 0, :], qe[:, 0, :], qe[:, 1, :])

            def fin(blk, half, off=0, tl=C):
                """SiLU (PSUM for PE chunks, SBUF for DVE chunks) + dense
                store of the finished [128, tl] chunk."""
                r0 = half * DH + blk * 128
                o = pool.tile([128, tl], MID_DT, tag="o", bufs=6)
                if (blk, half, off) in ps_of:
                    src = ps_of.pop((blk, half, off))[:]
                else:
                    src = qe_of.pop((blk, half, off))[:, 0, :]
                nc.scalar.activation(o[:], src, mybir.ActivationFunctionType.Silu)
                nc.gpsimd.dma_start(out=ot[r0 : r0 + 128, off : off + tl], in_=o[:])

            # Chunk schedule. First chunks of both streams split at 1024
            # (their x rows load in two pieces) for an early start; last
            # chunks split at 1024 for a fine-grained drain; block 6 rides
            # early in the PE stream.
            P = [(1, 0, 0, 1024), (1, 0, 1024, 1024), (6, 0, 0, C),
                 (3, 0, 0, C), (6, 1, 0, C), (5, 0, 0, C), (7, 0, 0, C),
                 (1, 1, 0, C), (3, 1, 0, C), (5, 1, 0, C),
                 (7, 1, 0, 1024), (7, 1, 1024, 1024)]
            E = [(0, 0, 0, 1024), (0, 0, 1024, 1024), (2, 0, 0, C),
                 (4, 0, 0, C), (0, 1, 0, C), (2, 1, 0, C),
                 (4, 1, 0, 1024), (4, 1, 1024, 1024)]

            # Loads in need-time order; split first rows keep both compute
            # streams' first chunks small and early.
            load(1, 0, 0, 1027)
            load(1, 0, 1024, XROW)
            load(0, 0, 0, 1027)
            load(0, 0, 1024, XROW)
            for blk, half in [(6, 0), (3, 0), (2, 0), (6, 1), (4, 0),
                              (5, 0), (7, 0), (0, 1), (1, 1), (3, 1),
                              (2, 1), (5, 1), (4, 1), (7, 1)]:
                load(blk, half)

            # Diags in engine dead time: block 1 on DVE (before its first
            # chain), block 6 on ACT (after warmup), 3/5/7 on DVE between
            # early chains -- each lands just before the PE stream's need.
            # Emission interleaves the two compute streams so every diag
            # write precedes its consuming matmuls (the Tile dependency
            # tracker follows emission order); within each engine the
            # emission order is exactly the intended execution order.
            diag_build("dve", 1)
            diag_build("act", 6)
            pe_mm(*P[0])
            pe_mm(*P[1])
            pe_mm(*P[2])
            dve_chain(*E[0])
            diag_build("dve", 3)
            pe_mm(*P[3])
            dve_chain(*E[1])
            diag_build("dve", 5)
            diag_build("dve", 7)
            for ch in P[4:]:
                pe_mm(*ch)
            for ch in E[2:]:
                dve_chain(*ch)

            # ACT + store order matched to expected completion times:
            # PE chunks ~3.9us apart, DVE chunks ~6.4us apart.
            fin_order = [
                P[0], P[1], E[0], P[2], E[1], P[3], E[2], P[4], P[5],
                E[3], P[6], P[7], E[4], P[8], E[5], P[9], P[10], E[6],
                P[11], E[7],
            ]
            for ch in fin_order:
                fin(*ch)
    nc.compile()
    return nc


def _shard_inputs(x, w):
    in_maps = []
    dg = np.eye(128, dtype=np.float16)
    for core in range(N_CORES):
        b, half = divmod(core, 2)
        d0 = half * DH
        xp = np.zeros((DH, PAD + L), dtype=np.float16)
        xp[:, PAD:] = x[b, :, d0 : d0 + DH].T.astype(np.float16)
        xt = np.zeros((2 * DH, XPITCH), dtype=np.float16)
        xt[:DH, 0:XROW] = xp[:, 0:XROW]
        xt[DH:, 0:XROW] = xp[:, C : C + XROW]
        # w rows for this shard, rearranged so partition p holds the K
        # weights of channel blk*128 + p at free cols [blk*K, blk*K + K)
        w_sh = w[d0 : d0 + DH].reshape(NBLK, 128, K)
        wt = (
            w_sh.transpose(1, 0, 2).reshape(128, NBLK * K).astype(np.float32)
        )
        in_maps.append(
            {
                "xt": np.ascontiguousarray(xt),
                "wt": np.ascontiguousarray(wt),
                "dg": dg,
            }
        )
    return in_maps


def kernel(x, w):
    x = np.asarray(x, dtype=np.float32)
    w = np.asarray(w, dtype=np.float32)
    assert x.shape == (B, L, D) and w.shape == (D, K)

    if "nc" not in _cache:
        _cache["nc"] = _build_bass()
    nc = _cache["nc"]

    in_maps = _shard_inputs(x, w)
    res = None
    for attempt in range(3):
        try:
            res = run_bass_kernel_spmd(nc, in_maps, core_ids=list(range(N_CORES)))
            break
        except Exception:
            if attempt == 2:
                raise
    _cache["last_results"] = res

    out = np.empty((B, L, D), dtype=np.float32)
    for core in range(N_CORES):
        b, half = divmod(core, 2)
        d0 = half * DH
        o3 = res.results[core]["ot"].reshape(2, DH, C)
        full = np.concatenate([o3[0], o3[1]], axis=1)  # [DH, L]
        out[b, :, d0 : d0 + DH] = full.T.astype(np.float32)
    return out
